# revision 5
# baseline (speedup 1.0000x reference)
"""GAT (2-layer graph attention network) Bass kernel for 8 trn2 NeuronCores.

Sharding: core c owns node rows [512c, 512c+512) as softmax targets (i); the
j-side (all 4096 nodes) is processed in 32 chunks of 128 partitions.

Layer-1 projection h1 = x @ W1 and all rank-1 score tables are computed on the
HOST (exact fp32) and DMA'd in, so the device runs a single fused pass over
the 32 j-chunks with all 4 heads at once (no all-gather, no phases):

  heads 0,1 (ACT path):  P = patchedExp(s_src[i] + s_dst[j]) * mask
  heads 2,3 (MAX path):  P' = max(E1d[j], r[i]*E2d[j]) * mask
      via exp(lrelu(t))/exp(s_src) = max(exp(s_dst), exp(-.8 s_src)*exp(.2 s_dst)),
      softmax-invariant (the 1/exp(s_src[i]) factor cancels in alpha). The
      dual-op tensor_scalar (mult,max with two per-partition scalars) runs in
      the DVE 4x perf mode; mask multiplies are split between DVE and Pool.

Layer 2 projects h2 for the local shard and all-gathers one merged block
[h2 | 1 | s2_dst] per node ([4096, 66] bf16, single collective); layer-2
exponentials run on ACT while the gather is in flight where possible.
Softmax reciprocals use rec = patchedExp(-5*ln(K*den)) = 1/(K*den) (the
patched table computes exp(0.2 x) for x<0). Exp is pinned to the
natural_log_exp_and_others table set so Exp/Ln share one table load.
"""

import os

import numpy as np

N, FIN, HID, H, D1, C = 4096, 512, 256, 4, 64, 64
NCORES = 8
SH = N // NCORES          # 512 local nodes per core
NB = N // 128             # 32 j-chunks
OWN = 4                   # own j-blocks per core
NEG = 0.2
AUGH = D1 + 1             # 65 per head
AUG = AUGH * H            # 260
W2C = C + 2               # [h2 | 1 | s2dst]
KREC = 32.0               # reciprocal pre-scale (keeps ln(K*den) in (0, 17))
POOL_MASK = frozenset((1, 3, 5, 7, 9, 11, 13, 15, 17, 19, 21, 23))

_CACHED = {}


def _make_act_root(alpha=NEG):
    """Patch the neuron ACT tables so Exp computes g(x)=exp(lrelu(x)).

    Bucket entries are [d0,d1,d2,d3,x0,0,0,0] fp32 cubics evaluated as
    y = d0+(x-x0)(d1+(x-x0)(d2+(x-x0)d3)). For exp buckets centered at
    x0<0 we substitute the Taylor cubic of exp(alpha*x) at the same
    center. Ln buckets are untouched.
    """
    import json
    import shutil
    import tempfile

    from neuronxcc.driver.Job import Job
    from neuronxcc.driver.jobs.support.FindActInfo import findActInfoFile

    src_dir = os.path.dirname(findActInfoFile(Job.getPackageDir(), "gen3"))
    dst = tempfile.mkdtemp(prefix="gat_act_root_")
    for f in os.listdir(src_dir):
        shutil.copy(os.path.join(src_dir, f), os.path.join(dst, f))
        os.chmod(os.path.join(dst, f), 0o644)
    for set_name in ("exp_and_others", "natural_log_exp_and_others",
                     "exp_and_friends"):
        meta = json.load(open(os.path.join(dst, f"{set_name}.json")))
        start = meta["func_to_bkt_start_idx"].get("exp")
        if start is None:
            continue
        nxt = [s for s in sorted(meta["func_to_bkt_start_idx"].values())
               if s > start]
        end = nxt[0] if nxt else meta["bkt_entry_cnt"]
        path = os.path.join(dst, f"{set_name}_bkt.bin")
        b = np.fromfile(path, dtype=np.float32).reshape(-1, 8).copy()
        for i in range(start, end):
            x0, d0 = float(b[i, 4]), float(b[i, 0])
            if x0 >= 0 or not np.isfinite(d0) or d0 <= 0:
                continue
            e = np.exp(alpha * x0)
            b[i, 0:4] = [e, alpha * e, alpha * alpha * e / 2.0,
                         alpha ** 3 * e / 6.0]
        b.tofile(path)
    return os.path.join(dst, "act_info.json")


def _pin_exp_table(act_root, mybir, bacc):
    """Make bacc's table-load pass see Exp only in the set that also holds
    Ln, so the whole kernel uses one ACT table load (no Exp<->Ln flips)."""
    import json

    with open(act_root) as f:
        info = json.load(f)
    tables = {}
    for ent in info["act_func_sets"]:
        fns = set()
        for v in ent["act"].keys():
            try:
                fns.add(mybir.ActivationFunctionType.from_pwp(v))
            except Exception:
                pass
        if ent["name"] != "natural_log_exp_and_others":
            fns.discard(mybir.ActivationFunctionType.Exp)
        tables[ent["name"]] = fns
    bacc.get_activation_tables = lambda arch: tables
    return tables


def _build_nc():
    act_root = _make_act_root()
    os.environ["BASS_ACT_ROOT_JSON_PATH"] = act_root
    import concourse.mybir as mybir
    import concourse.tile as tile
    from concourse import bacc

    _pin_exp_table(act_root, mybir, bacc)

    f32 = mybir.dt.float32
    bf16 = mybir.dt.bfloat16
    Af = mybir.ActivationFunctionType
    Alu = mybir.AluOpType

    nc = bacc.Bacc("TRN2", target_bir_lowering=False, debug=False,
                   num_devices=NCORES)

    h1a_d = nc.dram_tensor("h1a", [128, NB * AUG], bf16,
                           kind="ExternalInput").ap()
    mT_d = nc.dram_tensor("maskT", [N, SH], bf16, kind="ExternalInput").ap()
    srow_d = nc.dram_tensor("srow", [1, 2 * SH], f32,
                            kind="ExternalInput").ap()
    rrow_d = nc.dram_tensor("rrow", [1, 2 * SH], bf16,
                            kind="ExternalInput").ap()
    sdstT_d = nc.dram_tensor("sdstT", [128, NB, 2], f32,
                             kind="ExternalInput").ap()
    E1dT_d = nc.dram_tensor("E1dT", [128, NB, 2], f32,
                            kind="ExternalInput").ap()
    E2dT_d = nc.dram_tensor("E2dT", [128, NB, 2], f32,
                            kind="ExternalInput").ap()
    W2e_d = nc.dram_tensor("W2e", [HID, AUGH], bf16, kind="ExternalInput").ap()
    v2s_d = nc.dram_tensor("v2s", [HID, 1], bf16, kind="ExternalInput").ap()
    outT_d = nc.dram_tensor("outT", [C, SH], f32, kind="ExternalOutput").ap()

    with tile.TileContext(nc) as tc:
        with (tc.tile_pool(name="persist", bufs=1) as pp,
              tc.tile_pool(name="dram", bufs=1, space="DRAM") as dpool):
            # ---------------- persistent SBUF tiles -----------------------
            h1aug = pp.tile([128, NB, AUG], bf16)
            maskr = pp.tile([128, NB, SH], bf16)
            srowT = pp.tile([1, 2, SH], f32)
            rrowT = pp.tile([1, 2, SH], bf16)
            ssrcb2 = pp.tile([128, 2, SH], f32)
            rb2 = pp.tile([128, 2, SH], bf16)
            sdstT = pp.tile([128, NB, 2], f32)
            E1dT = pp.tile([128, NB, 2], f32)
            E2dT = pp.tile([128, NB, 2], f32)
            z1Tl = pp.tile([128, 2, SH], bf16)
            zrow = pp.tile([128, 2, SH], f32)
            W2sb = pp.tile([128, 2, AUGH], bf16)
            v2sb = pp.tile([128, 2, 1], bf16)
            h2l = pp.tile([128, OWN, W2C], bf16)      # [h2 | 1 | s2dst]
            h2all = pp.tile([128, NB, W2C], bf16)
            s2dstf = pp.tile([128, NB, 1], f32)
            pexf = pp.tile([128, NB, SH], bf16)
            s2srow = pp.tile([1, SH], f32)
            s2srcb = pp.tile([128, SH], f32)
            ones_col = pp.tile([128, 1], bf16)
            rec2row = pp.tile([1, SH], f32)

            ag_in = dpool.tile([OWN * 128, W2C], bf16)
            ag_out = dpool.tile([N, W2C], bf16, addr_space="Shared")

            # ---------------- input DMAs ----------------------------------
            h1v_d = h1a_d.rearrange("p (jc c) -> p jc c", c=AUG)
            for q in range(4):
                nc.sync.dma_start(h1aug[:, 8 * q:8 * (q + 1), :],
                                  h1v_d[:, 8 * q:8 * (q + 1), :])
            nc.sync.dma_start(srowT[:], srow_d.rearrange("o (u i) -> o u i",
                                                         u=2))
            nc.sync.dma_start(rrowT[:], rrow_d.rearrange("o (u i) -> o u i",
                                                         u=2))
            nc.sync.dma_start(sdstT[:], sdstT_d)
            nc.sync.dma_start(E1dT[:], E1dT_d)
            nc.sync.dma_start(E2dT[:], E2dT_d)
            nc.gpsimd.partition_broadcast(ssrcb2[:], srowT[:])
            nc.gpsimd.partition_broadcast(rb2[:], rrowT[:])
            nc.vector.memset(ones_col[:], 1.0)
            for jc in range(NB):
                nc.sync.dma_start(maskr[:, jc, :],
                                  mT_d[jc * 128:(jc + 1) * 128, :])
            for kc in range(2):
                nc.sync.dma_start(W2sb[:, kc, :],
                                  W2e_d[kc * 128:(kc + 1) * 128, :])
                nc.sync.dma_start(v2sb[:, kc, :],
                                  v2s_d[kc * 128:(kc + 1) * 128, :])

            # ---------------- layer 1: fused 4-head pass ------------------
            with tc.tile_pool(name="l1ps", bufs=1, space="PSUM") as l1ps:
                o1 = l1ps.tile([AUGH, H, SH], f32, tag="o1")

                with tc.tile_pool(name="work", bufs=8) as wp:
                    for jc in range(NB):
                        mb = maskr[:, jc, :].unsqueeze(1).to_broadcast(
                            (128, 2, SH))
                        pa = wp.tile([128, 2, SH], bf16, tag="pa")
                        for u in range(2):
                            nc.scalar.activation(
                                pa[:, u, :], ssrcb2[:, u, :], Af.Exp,
                                bias=sdstT[:, jc, u:u + 1])
                        pam = wp.tile([128, 2, SH], bf16, tag="pam")
                        eng = nc.gpsimd if jc in POOL_MASK else nc.vector
                        eng.tensor_mul(pam[:], pa[:], mb)
                        px = wp.tile([128, 2, SH], bf16, tag="px")
                        for u in range(2):
                            nc.vector.tensor_scalar(
                                px[:, u, :], rb2[:, u, :],
                                E2dT[:, jc, u:u + 1], E1dT[:, jc, u:u + 1],
                                Alu.mult, Alu.max)
                        pxm = wp.tile([128, 2, SH], bf16, tag="pxm")
                        nc.vector.tensor_mul(pxm[:], px[:], mb)
                        for h in range(H):
                            src = pam if h < 2 else pxm
                            nc.tensor.matmul(
                                o1[:, h, :],
                                h1aug[:, jc, AUGH * h:AUGH * (h + 1)],
                                src[:, h % 2, :],
                                start=(jc == 0), stop=(jc == NB - 1))

                # ---------------- layer 1 epilogue: softmax + ELU ---------
                with tc.tile_pool(name="fin1", bufs=1) as fin:
                    lnv = fin.tile([1, H, SH], f32, tag="lnv")
                    nc.scalar.activation(lnv[:], o1[D1:D1 + 1, :, :], Af.Ln,
                                         scale=KREC)
                    rr = fin.tile([1, H, SH], f32, tag="rr")
                    nc.scalar.activation(rr[:], lnv[:], Af.Exp, scale=-5.0)
                    for h in range(H):
                        recb = fin.tile([D1, SH], f32, tag=f"recb{h}")
                        nc.gpsimd.partition_broadcast(recb[:], rr[:, h, :])
                        nc.vector.scalar_tensor_tensor(
                            zrow[D1 * (h % 2):D1 * (h % 2) + D1, h // 2, :],
                            o1[0:D1, h, :], KREC, recb[:],
                            op0=Alu.mult, op1=Alu.mult)
                    # ELU: max(z,0) + patchedExp(5*min(z,0)) - 1
                    rmax = fin.tile([128, 2, SH], f32, tag="rmax")
                    rmin = fin.tile([128, 2, SH], f32, tag="rmin")
                    ex = fin.tile([128, 2, SH], f32, tag="ex")
                    nc.vector.tensor_scalar_max(rmax[:], zrow[:], 0.0)
                    nc.vector.tensor_scalar_min(rmin[:], zrow[:], 0.0)
                    nc.scalar.activation(ex[:], rmin[:], Af.Exp, scale=5.0)
                    nc.vector.scalar_tensor_tensor(
                        z1Tl[:], ex[:], -1.0, rmax[:],
                        op0=Alu.add, op1=Alu.add)

            # ---------------- layer 2: local projections + one gather -----
            with tc.tile_pool(name="s2ps", bufs=2, space="PSUM") as s2ps:
                s2p = s2ps.tile([1, SH], f32, tag="s2p", bufs=1)
                for kc in range(2):
                    nc.tensor.matmul(s2p[:], v2sb[:, kc, :], z1Tl[:, kc, :],
                                     start=(kc == 0), stop=(kc == 1))
                nc.vector.tensor_copy(s2srow[:], s2p[:])
                nc.gpsimd.partition_broadcast(s2srcb[:], s2srow[:])

                nc.vector.tensor_copy(
                    h2l[:, :, C:C + 1],
                    ones_col[:].unsqueeze(1).to_broadcast((128, OWN, 1)))
                s2dp = s2ps.tile([128, OWN], f32, tag="s2dp", bufs=1)
                for k in range(OWN):
                    for kc in range(2):
                        nc.tensor.matmul(
                            s2dp[:, k:k + 1],
                            z1Tl[:, kc, k * 128:(k + 1) * 128],
                            W2sb[:, kc, C:C + 1],
                            start=(kc == 0), stop=(kc == 1))
                nc.vector.tensor_copy(h2l[:, :, C + 1:C + 2],
                                      s2dp[:].unsqueeze(2))
                for k in range(OWN):
                    h2p = s2ps.tile([128, AUGH], f32, tag="h2p")
                    for kc in range(2):
                        nc.tensor.matmul(
                            h2p[:], z1Tl[:, kc, k * 128:(k + 1) * 128],
                            W2sb[:, kc, :], start=(kc == 0), stop=(kc == 1))
                    nc.vector.tensor_copy(h2l[:, k, 0:C], h2p[:, 0:C])
                    nc.sync.dma_start(ag_in[k * 128:(k + 1) * 128, :],
                                      h2l[:, k, :])

            nc.gpsimd.collective_compute(
                "AllGather", Alu.bypass,
                replica_groups=[list(range(NCORES))],
                ins=[ag_in[:].opt()], outs=[ag_out[:].opt()])
            agv = ag_out[:].rearrange("(jc p) c -> p jc c", p=128)
            nc.sync.dma_start(h2all[:], agv)
            nc.vector.tensor_copy(s2dstf[:], h2all[:, :, C + 1:C + 2])

            # ---------------- layer 2: masked softmax + aggregation -------
            with tc.tile_pool(name="aggps2", bufs=1, space="PSUM") as aggps2:
                o2 = aggps2.tile([AUGH, SH], f32, tag="o2")
                for jc in range(NB):
                    nc.scalar.activation(pexf[:, jc, :], s2srcb[:], Af.Exp,
                                         bias=s2dstf[:, jc, :])
                with tc.tile_pool(name="work2", bufs=8) as wp2:
                    for q in range(NB // 2):
                        pt = wp2.tile([128, 2, SH], bf16, tag="ptb")
                        nc.vector.tensor_mul(pt[:], pexf[:, 2 * q:2 * q + 2, :],
                                             maskr[:, 2 * q:2 * q + 2, :])
                        for r in range(2):
                            jc = 2 * q + r
                            nc.tensor.matmul(o2[:], h2all[:, jc, 0:AUGH],
                                             pt[:, r, :],
                                             start=(jc == 0),
                                             stop=(jc == NB - 1))

                with tc.tile_pool(name="fin2", bufs=1) as fin2:
                    u2 = fin2.tile([1, SH], f32, tag="u2")
                    nc.scalar.activation(u2[:], o2[D1:D1 + 1, :], Af.Ln,
                                         scale=KREC)
                    nc.scalar.activation(rec2row[:], u2[:], Af.Exp, scale=-5.0)
                    recb2 = fin2.tile([C, SH], f32, tag="recb2")
                    nc.gpsimd.partition_broadcast(recb2[:], rec2row[:])
                    outsb = fin2.tile([C, SH], f32, tag="outsb")
                    nc.vector.scalar_tensor_tensor(
                        outsb[:], o2[0:D1, :], KREC, recb2[:],
                        op0=Alu.mult, op1=Alu.mult)
                    nc.sync.dma_start(outT_d, outsb[:])

    nc.compile()
    return nc


def _get_nc():
    if "nc" not in _CACHED:
        _CACHED["nc"] = _build_nc()
    return _CACHED["nc"]


def _prep_in_maps(x, A, W1, a1_src, a1_dst, W2, a2_src, a2_dst):
    import ml_dtypes
    f = np.float32
    bf = ml_dtypes.bfloat16
    h1 = (x.astype(f) @ W1.astype(f)).reshape(N, H, D1)
    s_src = np.einsum("nhd,hd->nh", h1, a1_src).astype(f)   # [N, H]
    s_dst = np.einsum("nhd,hd->nh", h1, a1_dst).astype(f)   # [N, H]

    h1aug = np.empty((N, H, AUGH), f)
    h1aug[:, :, :D1] = h1
    h1aug[:, :, D1] = 1.0
    # device layout [128, NB, AUG]
    h1a_dev = np.ascontiguousarray(
        h1aug.reshape(NB, 128, AUG).transpose(1, 0, 2)).astype(bf)
    h1a_dev = h1a_dev.reshape(128, NB * AUG)

    def jlay(a):                                   # [N, 2] -> [128, NB, 2]
        return np.ascontiguousarray(
            a.reshape(NB, 128, 2).transpose(1, 0, 2)).astype(f)

    sdstT = jlay(s_dst[:, 0:2])
    E1dT = jlay(np.exp(s_dst[:, 2:4]))
    E2dT = jlay(np.exp(NEG * s_dst[:, 2:4]))
    W2e = np.concatenate([W2, W2 @ a2_dst.T], axis=1).astype(bf)
    v2s = (W2 @ a2_src.T).astype(bf)

    in_maps = []
    for c in range(NCORES):
        sl = slice(c * SH, (c + 1) * SH)
        srow = np.ascontiguousarray(
            s_src[sl, 0:2].T).reshape(1, 2 * SH).astype(f)
        rrow = np.ascontiguousarray(
            np.exp(-(1.0 - NEG) * s_src[sl, 2:4]).T).reshape(
                1, 2 * SH).astype(bf)
        in_maps.append({
            "h1a": h1a_dev,
            "maskT": np.ascontiguousarray((A[sl, :] > 0).T).astype(bf),
            "srow": srow,
            "rrow": rrow,
            "sdstT": sdstT,
            "E1dT": E1dT,
            "E2dT": E2dT,
            "W2e": W2e,
            "v2s": v2s,
        })
    return in_maps


def kernel(x, A, W1, a1_src, a1_dst, W2, a2_src, a2_dst, _want_results=False):
    from concourse.bass_utils import run_bass_kernel_spmd

    nc = _get_nc()
    in_maps = _prep_in_maps(np.asarray(x), np.asarray(A), np.asarray(W1),
                            np.asarray(a1_src), np.asarray(a1_dst),
                            np.asarray(W2), np.asarray(a2_src),
                            np.asarray(a2_dst))
    trace = bool(int(os.environ.get("GAT_TRACE", "0")))
    res = run_bass_kernel_spmd(nc, in_maps, core_ids=list(range(NCORES)),
                               trace=trace)
    out = np.empty((N, C), np.float32)
    for c in range(NCORES):
        out[c * SH:(c + 1) * SH, :] = res.results[c]["outT"].T
    if _want_results:
        return out, res
    return out


# revision 10
# speedup vs baseline: 1.3734x; 1.3734x over previous
"""GAT (2-layer graph attention network) Bass kernel for 8 trn2 NeuronCores.

Sharding: core c owns node rows [512c, 512c+512) as softmax targets (i); the
j-side (all 4096 nodes) is processed in 32 chunks of 128 partitions.

Layer-1 projection h1 = x @ W1 and all rank-1 score tables are computed on the
HOST (exact fp32) and DMA'd in, so the device runs a single fused pass over
the 32 j-chunks with all 4 heads at once (no all-gather, no phases):

  heads 0,1 (ACT path):  P = patchedExp(s_src[i] + s_dst[j]) * mask
  heads 2,3 (MAX path):  P' = max(E1d[j], r[i]*E2d[j]) * mask
      via exp(lrelu(t))/exp(s_src) = max(exp(s_dst), exp(-.8 s_src)*exp(.2 s_dst)),
      softmax-invariant (the 1/exp(s_src[i]) factor cancels in alpha). The
      dual-op tensor_scalar (mult,max with two per-partition scalars) runs in
      the DVE 4x perf mode; mask multiplies are split between DVE and Pool.

Layer 2 projects h2 for the local shard and all-gathers one merged block
[h2 | 1 | s2_dst] per node ([4096, 66] bf16, single collective); layer-2
exponentials run on ACT while the gather is in flight where possible.
Softmax reciprocals use rec = patchedExp(-5*ln(K*den)) = 1/(K*den) (the
patched table computes exp(0.2 x) for x<0). Exp is pinned to the
natural_log_exp_and_others table set so Exp/Ln share one table load.
"""

import os

import numpy as np

N, FIN, HID, H, D1, C = 4096, 512, 256, 4, 64, 64
NCORES = 8
SH = N // NCORES          # 512 local nodes per core
NB = N // 128             # 32 j-chunks
OWN = 4                   # own j-blocks per core
NEG = 0.2
AUGH = D1 + 1             # 65 per head
AUG = AUGH * H            # 260
W2C = C + 2               # [h2 | 1 | s2dst]
KREC = 32.0               # reciprocal pre-scale (keeps ln(K*den) in (0, 17))
POOL_MASK = frozenset(range(1, 29, 2)) | {2, 6, 10, 14}

_CACHED = {}


def _make_act_root(alpha=NEG):
    """Patch the neuron ACT tables so Exp computes g(x)=exp(lrelu(x)).

    Bucket entries are [d0,d1,d2,d3,x0,0,0,0] fp32 cubics evaluated as
    y = d0+(x-x0)(d1+(x-x0)(d2+(x-x0)d3)). For exp buckets centered at
    x0<0 we substitute the Taylor cubic of exp(alpha*x) at the same
    center. Ln buckets are untouched.
    """
    import json
    import shutil
    import tempfile

    from neuronxcc.driver.Job import Job
    from neuronxcc.driver.jobs.support.FindActInfo import findActInfoFile

    src_dir = os.path.dirname(findActInfoFile(Job.getPackageDir(), "gen3"))
    dst = tempfile.mkdtemp(prefix="gat_act_root_")
    for f in os.listdir(src_dir):
        shutil.copy(os.path.join(src_dir, f), os.path.join(dst, f))
        os.chmod(os.path.join(dst, f), 0o644)
    for set_name in ("exp_and_others", "natural_log_exp_and_others",
                     "exp_and_friends"):
        meta = json.load(open(os.path.join(dst, f"{set_name}.json")))
        start = meta["func_to_bkt_start_idx"].get("exp")
        if start is None:
            continue
        nxt = [s for s in sorted(meta["func_to_bkt_start_idx"].values())
               if s > start]
        end = nxt[0] if nxt else meta["bkt_entry_cnt"]
        path = os.path.join(dst, f"{set_name}_bkt.bin")
        b = np.fromfile(path, dtype=np.float32).reshape(-1, 8).copy()
        for i in range(start, end):
            x0, d0 = float(b[i, 4]), float(b[i, 0])
            if x0 >= 0 or not np.isfinite(d0) or d0 <= 0:
                continue
            e = np.exp(alpha * x0)
            b[i, 0:4] = [e, alpha * e, alpha * alpha * e / 2.0,
                         alpha ** 3 * e / 6.0]
        b.tofile(path)
    return os.path.join(dst, "act_info.json")


def _pin_exp_table(act_root, mybir, bacc):
    """Make bacc's table-load pass see Exp only in the set that also holds
    Ln, so the whole kernel uses one ACT table load (no Exp<->Ln flips)."""
    import json

    with open(act_root) as f:
        info = json.load(f)
    tables = {}
    for ent in info["act_func_sets"]:
        fns = set()
        for v in ent["act"].keys():
            try:
                fns.add(mybir.ActivationFunctionType.from_pwp(v))
            except Exception:
                pass
        if ent["name"] != "natural_log_exp_and_others":
            fns.discard(mybir.ActivationFunctionType.Exp)
        tables[ent["name"]] = fns
    bacc.get_activation_tables = lambda arch: tables
    return tables


def _build_nc():
    act_root = _make_act_root()
    os.environ["BASS_ACT_ROOT_JSON_PATH"] = act_root
    import concourse.mybir as mybir
    import concourse.tile as tile
    from concourse import bacc

    _pin_exp_table(act_root, mybir, bacc)

    f32 = mybir.dt.float32
    bf16 = mybir.dt.bfloat16
    Af = mybir.ActivationFunctionType
    Alu = mybir.AluOpType

    nc = bacc.Bacc("TRN2", target_bir_lowering=False, debug=False,
                   num_devices=NCORES)

    h1a_d = nc.dram_tensor("h1a", [128, NB * AUG], bf16,
                           kind="ExternalInput").ap()
    mT_d = nc.dram_tensor("maskT", [N, SH], bf16, kind="ExternalInput").ap()
    srow_d = nc.dram_tensor("srow", [1, 2 * SH], f32,
                            kind="ExternalInput").ap()
    rrow_d = nc.dram_tensor("rrow", [1, 2 * SH], bf16,
                            kind="ExternalInput").ap()
    sdstT_d = nc.dram_tensor("sdstT", [128, NB, 2], f32,
                             kind="ExternalInput").ap()
    E1dT_d = nc.dram_tensor("E1dT", [128, NB, 2], f32,
                            kind="ExternalInput").ap()
    E2dT_d = nc.dram_tensor("E2dT", [128, NB, 2], f32,
                            kind="ExternalInput").ap()
    W2e_d = nc.dram_tensor("W2e", [HID, AUGH], bf16, kind="ExternalInput").ap()
    v2s_d = nc.dram_tensor("v2s", [HID, 1], bf16, kind="ExternalInput").ap()
    outT_d = nc.dram_tensor("outT", [C, SH], f32, kind="ExternalOutput").ap()

    with tile.TileContext(nc) as tc:
        with (tc.tile_pool(name="persist", bufs=1) as pp,
              tc.tile_pool(name="dram", bufs=1, space="DRAM") as dpool):
            # ---------------- persistent SBUF tiles -----------------------
            h1aug = pp.tile([128, NB, AUG], bf16)
            maskr = pp.tile([128, NB, SH], bf16)
            srowT = pp.tile([1, 2, SH], f32)
            rrowT = pp.tile([1, 2, SH], bf16)
            ssrcb2 = pp.tile([128, 2, SH], f32)
            rb2 = pp.tile([128, 2, SH], bf16)
            sdstT = pp.tile([128, NB, 2], f32)
            E1dT = pp.tile([128, NB, 2], f32)
            E2dT = pp.tile([128, NB, 2], f32)
            z1Tl = pp.tile([128, 2, SH], bf16)
            zrow = pp.tile([128, 2, SH], f32)
            W2sb = pp.tile([128, 2, AUGH], bf16)
            v2sb = pp.tile([128, 2, 1], bf16)
            h2l = pp.tile([128, OWN, W2C], bf16)      # [h2 | 1 | s2dst]
            h2all = pp.tile([128, NB, W2C], bf16)
            s2dstf = pp.tile([128, NB, 1], f32)
            pexf = pp.tile([128, NB, SH], bf16)
            s2srow = pp.tile([1, SH], f32)
            s2srcb = pp.tile([128, SH], f32)
            ones_col = pp.tile([128, 1], bf16)
            rec2row = pp.tile([1, SH], f32)

            ag_in = dpool.tile([OWN * 128, W2C], bf16)
            ag_out = dpool.tile([N, W2C], bf16, addr_space="Shared")
            sync_in = dpool.tile([1, 64], bf16)
            sync_out = dpool.tile([NCORES, 64], bf16, addr_space="Shared")

            # Tiny dummy collective issued first: the runtime's one-time
            # inter-core barrier rendezvous happens at the FIRST collective
            # trigger, so issue one immediately to absorb launch skew during
            # the prologue instead of stalling the real gather at the end.
            scr = pp.tile([1, 64], bf16)
            nc.vector.memset(scr[:], 0.0)
            nc.sync.dma_start(sync_in[:], scr[:])
            nc.gpsimd.collective_compute(
                "AllGather", Alu.bypass,
                replica_groups=[list(range(NCORES))],
                ins=[sync_in[:].opt()], outs=[sync_out[:].opt()])

            # ---------------- input DMAs ----------------------------------
            h1v_d = h1a_d.rearrange("p (jc c) -> p jc c", c=AUG)
            for q in range(4):
                nc.sync.dma_start(h1aug[:, 8 * q:8 * (q + 1), :],
                                  h1v_d[:, 8 * q:8 * (q + 1), :])
            nc.sync.dma_start(srowT[:], srow_d.rearrange("o (u i) -> o u i",
                                                         u=2))
            nc.sync.dma_start(rrowT[:], rrow_d.rearrange("o (u i) -> o u i",
                                                         u=2))
            nc.sync.dma_start(sdstT[:], sdstT_d)
            nc.sync.dma_start(E1dT[:], E1dT_d)
            nc.sync.dma_start(E2dT[:], E2dT_d)
            nc.gpsimd.partition_broadcast(ssrcb2[:], srowT[:])
            nc.gpsimd.partition_broadcast(rb2[:], rrowT[:])
            nc.vector.memset(ones_col[:], 1.0)
            for jc in range(NB):
                nc.sync.dma_start(maskr[:, jc, :],
                                  mT_d[jc * 128:(jc + 1) * 128, :])
            for kc in range(2):
                nc.sync.dma_start(W2sb[:, kc, :],
                                  W2e_d[kc * 128:(kc + 1) * 128, :])
                nc.sync.dma_start(v2sb[:, kc, :],
                                  v2s_d[kc * 128:(kc + 1) * 128, :])

            # ---------------- layer 1: fused 4-head pass ------------------
            with tc.tile_pool(name="l1ps", bufs=1, space="PSUM") as l1ps:
                o1 = l1ps.tile([AUGH, H, SH], f32, tag="o1")

                with tc.tile_pool(name="work", bufs=8) as wp:
                    # ACT-path exponentials run one chunk ahead so the
                    # in-order DVE queue never stalls on same-chunk ACTs.
                    pa_h = {}

                    def emit_pa(jc):
                        pa_h[jc] = wp.tile([128, 2, SH], bf16, tag="pa",
                                           name=f"pa{jc}")
                        for u in range(2):
                            nc.scalar.activation(
                                pa_h[jc][:, u, :], ssrcb2[:, u, :], Af.Exp,
                                bias=sdstT[:, jc, u:u + 1])

                    emit_pa(0)
                    for jc in range(NB):
                        mb = maskr[:, jc, :].unsqueeze(1).to_broadcast(
                            (128, 2, SH))
                        if jc + 1 < NB:
                            emit_pa(jc + 1)
                        px = wp.tile([128, 2, SH], bf16, tag="px")
                        for u in range(2):
                            nc.vector.tensor_scalar(
                                px[:, u, :], rb2[:, u, :],
                                E2dT[:, jc, u:u + 1], E1dT[:, jc, u:u + 1],
                                Alu.mult, Alu.max)
                        pxm = wp.tile([128, 2, SH], bf16, tag="pxm")
                        nc.vector.tensor_mul(pxm[:], px[:], mb)
                        pam = wp.tile([128, 2, SH], bf16, tag="pam")
                        eng = nc.gpsimd if jc in POOL_MASK else nc.vector
                        eng.tensor_mul(pam[:], pa_h.pop(jc)[:], mb)
                        for h in (2, 3, 0, 1):
                            src = pam if h < 2 else pxm
                            nc.tensor.matmul(
                                o1[:, h, :],
                                h1aug[:, jc, AUGH * h:AUGH * (h + 1)],
                                src[:, h % 2, :],
                                start=(jc == 0), stop=(jc == NB - 1))

                # ---------------- layer 1 epilogue: softmax + ELU ---------
                with tc.tile_pool(name="fin1", bufs=1) as fin:
                    lnv = fin.tile([1, H, SH], f32, tag="lnv")
                    nc.scalar.activation(lnv[:], o1[D1:D1 + 1, :, :], Af.Ln,
                                         scale=KREC)
                    rr = fin.tile([1, H, SH], f32, tag="rr")
                    nc.scalar.activation(rr[:], lnv[:], Af.Exp, scale=-5.0)
                    for h in range(H):
                        recb = fin.tile([D1, SH], f32, tag=f"recb{h}")
                        nc.gpsimd.partition_broadcast(recb[:], rr[:, h, :])
                        nc.vector.scalar_tensor_tensor(
                            zrow[D1 * (h % 2):D1 * (h % 2) + D1, h // 2, :],
                            o1[0:D1, h, :], KREC, recb[:],
                            op0=Alu.mult, op1=Alu.mult)
                    # ELU: max(z,0) + patchedExp(5*min(z,0)) - 1
                    rmax = fin.tile([128, 2, SH], f32, tag="rmax")
                    rmin = fin.tile([128, 2, SH], f32, tag="rmin")
                    ex = fin.tile([128, 2, SH], f32, tag="ex")
                    nc.vector.tensor_scalar_max(rmax[:], zrow[:], 0.0)
                    nc.vector.tensor_scalar_min(rmin[:], zrow[:], 0.0)
                    nc.scalar.activation(ex[:], rmin[:], Af.Exp, scale=5.0)
                    nc.vector.scalar_tensor_tensor(
                        z1Tl[:], ex[:], -1.0, rmax[:],
                        op0=Alu.add, op1=Alu.add)

            # ---------------- layer 2: local projections + one gather -----
            with tc.tile_pool(name="s2ps", bufs=2, space="PSUM") as s2ps:
                s2p = s2ps.tile([1, SH], f32, tag="s2p", bufs=1)
                for kc in range(2):
                    nc.tensor.matmul(s2p[:], v2sb[:, kc, :], z1Tl[:, kc, :],
                                     start=(kc == 0), stop=(kc == 1))
                nc.vector.tensor_copy(s2srow[:], s2p[:])
                nc.gpsimd.partition_broadcast(s2srcb[:], s2srow[:])

                nc.vector.tensor_copy(
                    h2l[:, :, C:C + 1],
                    ones_col[:].unsqueeze(1).to_broadcast((128, OWN, 1)))
                s2dp = s2ps.tile([128, OWN], f32, tag="s2dp", bufs=1)
                for k in range(OWN):
                    for kc in range(2):
                        nc.tensor.matmul(
                            s2dp[:, k:k + 1],
                            z1Tl[:, kc, k * 128:(k + 1) * 128],
                            W2sb[:, kc, C:C + 1],
                            start=(kc == 0), stop=(kc == 1))
                nc.vector.tensor_copy(h2l[:, :, C + 1:C + 2],
                                      s2dp[:].unsqueeze(2))
                for k in range(OWN):
                    h2p = s2ps.tile([128, AUGH], f32, tag="h2p")
                    for kc in range(2):
                        nc.tensor.matmul(
                            h2p[:], z1Tl[:, kc, k * 128:(k + 1) * 128],
                            W2sb[:, kc, :], start=(kc == 0), stop=(kc == 1))
                    nc.vector.tensor_copy(h2l[:, k, 0:C], h2p[:, 0:C])
                    nc.sync.dma_start(ag_in[k * 128:(k + 1) * 128, :],
                                      h2l[:, k, :])

            nc.gpsimd.collective_compute(
                "AllGather", Alu.bypass,
                replica_groups=[list(range(NCORES))],
                ins=[ag_in[:].opt()], outs=[ag_out[:].opt()])
            agv = ag_out[:].rearrange("(jc p) c -> p jc c", p=128)
            nc.sync.dma_start(h2all[:], agv)
            nc.vector.tensor_copy(s2dstf[:], h2all[:, :, C + 1:C + 2])

            # ---------------- layer 2: masked softmax + aggregation -------
            with tc.tile_pool(name="aggps2", bufs=1, space="PSUM") as aggps2:
                o2 = aggps2.tile([AUGH, SH], f32, tag="o2")
                for jc in range(NB):
                    nc.scalar.activation(pexf[:, jc, :], s2srcb[:], Af.Exp,
                                         bias=s2dstf[:, jc, :])
                with tc.tile_pool(name="work2", bufs=8) as wp2:
                    for q in range(NB // 2):
                        pt = wp2.tile([128, 2, SH], bf16, tag="ptb")
                        nc.vector.tensor_mul(pt[:], pexf[:, 2 * q:2 * q + 2, :],
                                             maskr[:, 2 * q:2 * q + 2, :])
                        for r in range(2):
                            jc = 2 * q + r
                            nc.tensor.matmul(o2[:], h2all[:, jc, 0:AUGH],
                                             pt[:, r, :],
                                             start=(jc == 0),
                                             stop=(jc == NB - 1))

                with tc.tile_pool(name="fin2", bufs=1) as fin2:
                    u2 = fin2.tile([1, SH], f32, tag="u2")
                    nc.scalar.activation(u2[:], o2[D1:D1 + 1, :], Af.Ln,
                                         scale=KREC)
                    nc.scalar.activation(rec2row[:], u2[:], Af.Exp, scale=-5.0)
                    recb2 = fin2.tile([C, SH], f32, tag="recb2")
                    nc.gpsimd.partition_broadcast(recb2[:], rec2row[:])
                    outsb = fin2.tile([C, SH], f32, tag="outsb")
                    nc.vector.scalar_tensor_tensor(
                        outsb[:], o2[0:D1, :], KREC, recb2[:],
                        op0=Alu.mult, op1=Alu.mult)
                    nc.sync.dma_start(outT_d, outsb[:])

    nc.compile()
    return nc


def _get_nc():
    if "nc" not in _CACHED:
        _CACHED["nc"] = _build_nc()
    return _CACHED["nc"]


def _prep_in_maps(x, A, W1, a1_src, a1_dst, W2, a2_src, a2_dst):
    import ml_dtypes
    f = np.float32
    bf = ml_dtypes.bfloat16
    h1 = (x.astype(f) @ W1.astype(f)).reshape(N, H, D1)
    s_src = np.einsum("nhd,hd->nh", h1, a1_src).astype(f)   # [N, H]
    s_dst = np.einsum("nhd,hd->nh", h1, a1_dst).astype(f)   # [N, H]

    h1aug = np.empty((N, H, AUGH), f)
    h1aug[:, :, :D1] = h1
    h1aug[:, :, D1] = 1.0
    # device layout [128, NB, AUG]
    h1a_dev = np.ascontiguousarray(
        h1aug.reshape(NB, 128, AUG).transpose(1, 0, 2)).astype(bf)
    h1a_dev = h1a_dev.reshape(128, NB * AUG)

    def jlay(a):                                   # [N, 2] -> [128, NB, 2]
        return np.ascontiguousarray(
            a.reshape(NB, 128, 2).transpose(1, 0, 2)).astype(f)

    sdstT = jlay(s_dst[:, 0:2])
    E1dT = jlay(np.exp(s_dst[:, 2:4]))
    E2dT = jlay(np.exp(NEG * s_dst[:, 2:4]))
    W2e = np.concatenate([W2, W2 @ a2_dst.T], axis=1).astype(bf)
    v2s = (W2 @ a2_src.T).astype(bf)

    in_maps = []
    for c in range(NCORES):
        sl = slice(c * SH, (c + 1) * SH)
        srow = np.ascontiguousarray(
            s_src[sl, 0:2].T).reshape(1, 2 * SH).astype(f)
        rrow = np.ascontiguousarray(
            np.exp(-(1.0 - NEG) * s_src[sl, 2:4]).T).reshape(
                1, 2 * SH).astype(bf)
        in_maps.append({
            "h1a": h1a_dev,
            "maskT": np.ascontiguousarray((A[sl, :] > 0).T).astype(bf),
            "srow": srow,
            "rrow": rrow,
            "sdstT": sdstT,
            "E1dT": E1dT,
            "E2dT": E2dT,
            "W2e": W2e,
            "v2s": v2s,
        })
    return in_maps


def kernel(x, A, W1, a1_src, a1_dst, W2, a2_src, a2_dst, _want_results=False):
    from concourse.bass_utils import run_bass_kernel_spmd

    nc = _get_nc()
    in_maps = _prep_in_maps(np.asarray(x), np.asarray(A), np.asarray(W1),
                            np.asarray(a1_src), np.asarray(a1_dst),
                            np.asarray(W2), np.asarray(a2_src),
                            np.asarray(a2_dst))
    trace = bool(int(os.environ.get("GAT_TRACE", "0")))
    res = run_bass_kernel_spmd(nc, in_maps, core_ids=list(range(NCORES)),
                               trace=trace)
    out = np.empty((N, C), np.float32)
    for c in range(NCORES):
        out[c * SH:(c + 1) * SH, :] = res.results[c]["outT"].T
    if _want_results:
        return out, res
    return out
